# revision 36
# baseline (speedup 1.0000x reference)
"""Multi-head graph attention layer (GAT, no softmax) on 8 Trainium2 NeuronCores.

Row-shard the N=4096 nodes across the 8 cores (512 columns of P^T each).
Host precomputes Wh = h@W (bf16, [m, (h,o)] layout), the attention
projections s_h[n] = Wh.a1 and t_h[m] = Wh.a2, and the transposed,
slope-prescaled mask bigat02[m, n] = 0.2*BIG*(adj[n, m]-1).

Key factorization: in bf16, masked attention entries collapse to the
constant -0.2*BIG (~-9.007e15, matching the reference's -9e15 to ~8e-4),
so  P^T = bigat02 + prelu_0.2(s[n] + t[m])  exactly (the prelu leak on
masked entries is ~1e-15 relative).  The bigat02 term never touches a
vector engine: it enters PSUM through 4 full-width shared matmuls per
m-block on the otherwise-slack PE.  The prelu(s+t) term needs no mask
stream, so per (m-block, head) it is one op on one engine:
  - heads 0..4: ONE custom fused DVE instruction (GAT_PRELU_E) computes
    prelu(s-row-bcast + t-col-bcast) for all 5 heads (2560 elems @ 1/cyc)
  - heads 5..7: ACT Prelu with in_=s-rows, bias=t, alpha=0.2
PE accumulates the 8 per-head attention matmuls on top of the mask term
in 4 PSUM banks; elu + store at the end (split across ACT/DVE queues).

s-rows are broadcast to 128 partitions at startup via rank-1 ones
matmuls (DMA broadcast is ~10us slower); all big loads are chunked with
small leading chunks so m-block 0 starts ~6us after the preamble.
Steady state is DVE-bound at ~2.74us per m-block (custom op at its
1 elem/cycle floor); ACT runs ~2.6us, PE ~2.5us.
"""

import numpy as np
import ml_dtypes

N = 4096
IN_F = 512
OUT_F = 64
HEADS = 8
NCORES = 8
NS = N // NCORES          # 512 rows per core
MB = N // 128             # 32 m-blocks
HO = HEADS * OUT_F        # 512
BIG = float(np.float32(1.25 * 2.0**55))   # 0.2*BIG = 2^53 ~= 9.007e15
ALPHA = 0.2

# Heads 0.._NDVE-1 compute prelu(s+t) in one fused custom DVE instruction;
# the remaining heads use ACT Prelu with bias=t.  The additive-BIG mask is
# factored out entirely: P = bigat02 + prelu(s [+] t), and the bigat02 term
# goes through 4 shared full-width matmuls on the (underutilized) PE.
import os
_NDVE = int(os.environ.get("GAT_NDVE", "5"))

_CACHE = {}


def _register_op(name, spec, subdim):
    """Register a custom DVE op (idempotent), computing its uops sha."""
    import concourse.dve_ops as dops
    from concourse.dve_spec import lower, _has_src1
    from concourse.dve_uop import DveOpSpec

    if name in dops._SUB_OPCODE_FOR_NAME:
        for op in dops.OPS:
            if op.name == name:
                return op
        raise RuntimeError(f"{name} row taken but op missing")
    row = dops._CUSTOM_DVE_ROW_BASE + len(dops.OPS)
    shas = {}
    for ver in ("v3", "v4"):
        try:
            tmp = DveOpSpec(name=name, opcode=row, uops=lower(spec, ver=ver),
                            rd1_en=_has_src1(spec))
            shas[ver] = tmp.sha(ver)
        except Exception:
            pass
    op = dops.DveOp(name, spec, subdim=subdim, uops_sha=shas)
    dops.OPS.append(op)
    dops._SUB_OPCODE_FOR_NAME[name] = row
    dops.CUSTOM_DVE_SPECS[name] = spec
    return op


def _register_gat_prelu():
    """Fused prelu(in0 + in1 + s0) with slope imm2, one head-block."""
    from concourse.dve_spec import Spec, Src0, Src1, C0, C2, maxx

    def _ref(in0, in1, s0, s1, imm2):
        y = in0.astype(np.float32) + in1 + s0
        return np.maximum(y, y * imm2)

    y = Src0 + Src1 + C0
    return _register_op("GAT_PRELU_ANT", Spec(body=maxx(y, y * C2), reference=_ref),
                        subdim=False)


def _register_gat_prelu2():
    """Paired fused prelu over [P, 2, N] pages: page s gets bias s0 + s*s1
    (s0 = t of head A, s1 = t_B - t_A), slope imm2.  in1 is the concatenated
    s-rows of both heads as a flat [P, 2N] stream."""
    from concourse.dve_spec import Spec, Src0, Src1, C0, C1, C2, PageIdx, maxx

    def _ref(in0, in1, s0, s1, imm2):
        P = in0.shape[0]
        x0 = in0.astype(np.float32).reshape(P, 2, -1)
        x1 = in1.astype(np.float32).reshape(P, 2, -1)
        s0 = np.asarray(s0, np.float32).reshape(P, 1, 1)
        s1 = np.asarray(s1, np.float32).reshape(P, 1, 1)
        t = s0 + np.arange(2, dtype=np.float32)[None, :, None] * s1
        y = x0 + x1 + t
        return np.maximum(y, y * imm2).reshape(in0.shape)

    y = Src0 + Src1 + PageIdx(C0, C1)
    return _register_op("GAT_PRELU2_ANT",
                        Spec(body=maxx(y, y * C2), reference=_ref),
                        subdim=True)


def _register_gat_prelu_e():
    """prelu(in0 + in1) with slope s0: computes prelu_0.2(s[n] + t[m]) for a
    multi-head block, with in1 the per-head t column broadcast along the
    free dim.  Two tensor streams, one scalar slope."""
    from concourse.dve_spec import Spec, Src0, Src1, C0, maxx

    def _ref(in0, in1, s0, s1, imm2):
        y = in0.astype(np.float32) + in1
        return np.maximum(y, y * s0)

    y = Src0 + Src1
    return _register_op("GAT_PRELU_E_ANT",
                        Spec(body=maxx(y, y * C0), reference=_ref),
                        subdim=False)


def _build():
    import concourse.bass as bass
    import concourse.mybir as mybir
    import concourse.tile as tile
    from concourse import bacc

    gat_prelu_e = _register_gat_prelu_e()

    f32 = mybir.dt.float32
    bf16 = mybir.dt.bfloat16
    Alu = mybir.AluOpType
    Act = mybir.ActivationFunctionType

    nc = bacc.Bacc("TRN2", target_bir_lowering=False, debug=False,
                   num_devices=1)

    # host-prearranged [128, MB, x] layouts for straight contiguous DMA.
    # bigat is pre-scaled by the prelu slope: {-0.2*BIG, 0}.
    whb_d = nc.dram_tensor("whb", [128, MB, HO], bf16, kind="ExternalInput")
    bigat_d = nc.dram_tensor("bigat", [128, MB, NS], bf16,
                             kind="ExternalInput")
    srow = nc.dram_tensor("srow", [HEADS, NS], bf16, kind="ExternalInput")
    tpack = nc.dram_tensor("tpack", [128, MB, HEADS], f32,
                           kind="ExternalInput")
    tpackh = nc.dram_tensor("tpackh", [128, MB, HEADS], bf16,
                            kind="ExternalInput")
    outT = nc.dram_tensor("out", [HO, NS], f32, kind="ExternalOutput")

    nDV = _NDVE                 # heads 0..nDV-1 on the fused DVE op
    act_heads = list(range(nDV, HEADS))

    with tile.TileContext(nc) as tc:
        import contextlib
        with contextlib.ExitStack() as ctx:
            P1 = ctx.enter_context(tc.tile_pool(name="persist", bufs=1))
            pp = ctx.enter_context(tc.tile_pool(name="pp", bufs=6))
            iop = ctx.enter_context(tc.tile_pool(name="iop", bufs=2))
            hpp = ctx.enter_context(
                tc.tile_pool(name="hpp", bufs=1, space="PSUM"))

            alph = P1.tile([128, 1], f32)
            nc.gpsimd.memset(alph, ALPHA)

            # ---- upfront loads ----
            # big streaming chunks go first on the sync queue (small chunks
            # first so mb0 deps land fast); scalar-side loads ride gpsimd
            ones1 = P1.tile([1, 128], bf16)
            nc.gpsimd.memset(ones1, 1.0)
            srow1 = P1.tile([1, HEADS * NS], bf16)
            sr_ap = srow.ap()
            nc.gpsimd.dma_start(
                out=srow1,
                in_=bass.AP(tensor=sr_ap.tensor, offset=sr_ap.offset,
                            ap=[[HEADS * NS, 1], [1, HEADS * NS]]))
            whb = P1.tile([128, MB, HO], bf16)
            bigat = P1.tile([128, MB, NS], bf16)
            CHUNKS = [1, 1, 1, 1, 4, 4, 4, 4, 4, 4, 4]
            pos = 0
            for ch in CHUNKS:
                cs = slice(pos, pos + ch)
                nc.sync.dma_start(out=bigat[:, cs, :],
                                  in_=bigat_d.ap()[:, cs, :])
                nc.sync.dma_start(out=whb[:, cs, :],
                                  in_=whb_d.ap()[:, cs, :])
                pos += ch
            tsbh = P1.tile([128, MB, HEADS], bf16)  # t in bf16 (DVE in1)
            nc.gpsimd.dma_start(out=tsbh, in_=tpackh.ap())
            tsb = P1.tile([128, MB, HEADS], f32)    # t_h[m] per partition
            nc.gpsimd.dma_start(out=tsb, in_=tpack.ap())
            # s rows broadcast to 128 partitions via rank-1 ones matmuls
            # (much faster than a 128-way broadcast DMA); DVE copies the
            # fused-op heads, ACT the rest, so both start early
            sbc = P1.tile([128, HEADS, NS], bf16)   # s_h[n] bcast over parts
            with tc.tile_pool(name="bcp", bufs=4, space="PSUM") as bcp:
                for sg in range(HEADS):
                    sps = bcp.tile([128, NS], f32, tag="sps")
                    nc.tensor.matmul(sps, ones1,
                                     srow1[:, NS * sg:NS * (sg + 1)],
                                     start=True, stop=True)
                    if sg < nDV:
                        nc.vector.tensor_copy(sbc[:, sg, :], sps)
                    else:
                        nc.scalar.activation(sbc[:, sg, :], sps, Act.Prelu,
                                             bias=0.0, scale=1.0, alpha=1.0)

            # ---- PSUM accumulators: h'^T[(h,o), n], 2 heads per bank ----
            hp0 = hpp.tile([128, NS], f32, tag="hp0")
            hp1 = hpp.tile([128, NS], f32, tag="hp1")
            hp2 = hpp.tile([128, NS], f32, tag="hp2")
            hp3 = hpp.tile([128, NS], f32, tag="hp3")
            hps = [hp0, hp1, hp2, hp3]

            for mb in range(MB):
                # every 4th m-block shifts one head to ACT (it has slack)
                ndv = nDV - 1 if mb % 3 == 2 else nDV
                ah = list(range(ndv, HEADS))
                # shared mask matmuls: hps[q] (+)= whb_cols_q^T @ bigat02
                for q in range(4):
                    nc.tensor.matmul(
                        hps[q], whb[:, mb, 128 * q:128 * (q + 1)],
                        bigat[:, mb, :],
                        start=(mb == 0), stop=False,
                        skip_group_check=True)
                pc = pp.tile([128, HEADS, NS], bf16, tag="pc")
                # DVE: fused prelu(s + t) for heads 0..ndv-1, one instruction
                # (in1 = t columns broadcast along the free dim)
                tsl = tsbh[:, mb, 0:ndv]
                nc.vector._custom_dve(
                    gat_prelu_e, out=pc[:, 0:ndv, :],
                    in0=sbc[:, 0:ndv, :],
                    in1=bass.AP(tensor=tsl.tensor, offset=tsl.offset,
                                ap=[tsl.ap[0], [tsl.ap[-1][0], ndv],
                                    [0, NS]]),
                    s0=ALPHA)
                # ACT: prelu(s + t) via bias for the rest
                for hh in ah:
                    nc.scalar.activation(pc[:, hh, :], sbc[:, hh, :],
                                         Act.Prelu,
                                         bias=tsb[:, mb, hh:hh + 1],
                                         scale=1.0, alpha=alph[:, 0:1])
                # PE: per-head attention matmuls (accumulate onto mask term)
                for hh in list(range(ndv)) + ah:
                    po = 64 * (hh % 2)
                    nc.tensor.matmul(
                        hps[hh // 2][po:po + 64, :],
                        whb[:, mb, OUT_F * hh:OUT_F * (hh + 1)],
                        pc[:, hh, :],
                        start=False, stop=(mb == MB - 1),
                        skip_group_check=True)

            # ---- output: elu, store transposed (host untransposes).
            # q0/q1 chains lean on ACT, q2/q3 on DVE, so the tails overlap.
            for q in range(4):
                rpos = iop.tile([128, NS], f32, tag=f"rpos{q}")
                rneg = iop.tile([128, NS], f32, tag=f"rneg{q}")
                ex = iop.tile([128, NS], f32, tag=f"ex{q}")
                if q < 2:
                    nc.scalar.activation(rpos, hps[q], Act.Relu)
                    nc.scalar.activation(rneg, hps[q], Act.Relu, scale=-1.0)
                    nc.scalar.activation(ex, rneg, Act.Exp, scale=-1.0)
                else:
                    nc.vector.tensor_scalar(rpos, hps[q], 0.0, None, Alu.max)
                    nc.vector.tensor_scalar(rneg, hps[q], 0.0, None, Alu.min)
                    nc.scalar.activation(ex, rneg, Act.Exp)
                oo = iop.tile([128, NS], f32, tag=f"oo{q}")
                nc.vector.scalar_tensor_tensor(
                    out=oo, in0=rpos, scalar=-1.0, in1=ex,
                    op0=Alu.add, op1=Alu.add)
                engs = [nc.sync, nc.gpsimd, nc.scalar]
                th = NS // 4
                for pz in range(4):
                    lo, hi = th * pz, th * (pz + 1)
                    engs[(q + pz) % 3].dma_start(
                        out=outT.ap()[128 * q:128 * (q + 1), lo:hi],
                        in_=oo[:, lo:hi])

    nc.compile()
    return nc


def _prep_inputs(h, adj, W, a):
    bf = ml_dtypes.bfloat16
    # Wh[h, n, o] then column-major (h,o) concat -> [n, 64h+o]
    Wh = np.matmul(h[None, :, :], W)                       # [H, N, O] f32
    whb_no = Wh.transpose(1, 0, 2).reshape(N, HO)          # [N, HO]
    whb = np.ascontiguousarray(
        whb_no.reshape(MB, 128, HO).transpose(1, 0, 2)).astype(bf)
    a1 = a[:, :OUT_F, 0]                                   # [H, O] (s side)
    a2 = a[:, OUT_F:, 0]                                   # [H, O] (t side)
    s_full = np.matmul(Wh, a1[:, :, None])[:, :, 0]        # [H, N]
    t_full = np.matmul(Wh, a2[:, :, None])[:, :, 0]        # [H, N]
    tpack = np.ascontiguousarray(
        t_full.T.reshape(MB, 128, HEADS).transpose(1, 0, 2)).astype(
            np.float32)                                    # [128, MB, H]
    # mask pre-scaled by the prelu slope: {-0.2*BIG, 0}
    bigaT = ((adj.T.astype(np.float32) - 1.0) * (ALPHA * BIG)).astype(bf)

    in_maps = []
    for c in range(NCORES):
        rows = slice(c * NS, (c + 1) * NS)
        bslice = np.ascontiguousarray(
            bigaT[:, rows].reshape(MB, 128, NS).transpose(1, 0, 2))
        in_maps.append({
            "whb": whb,
            "bigat": bslice,
            "srow": np.ascontiguousarray(s_full[:, rows]).astype(bf),
            "tpack": tpack,
            "tpackh": tpack.astype(bf),
        })
    return in_maps


def _get_nc():
    if "nc" not in _CACHE:
        _CACHE["nc"] = _build()
    return _CACHE["nc"]


def kernel(h, adj, W, a, _trace=False, _trace_kwargs=None):
    from concourse.bass_utils import run_bass_kernel_spmd

    h = np.asarray(h, dtype=np.float32)
    adj = np.asarray(adj, dtype=np.int32)
    W = np.asarray(W, dtype=np.float32)
    a = np.asarray(a, dtype=np.float32)

    nc = _get_nc()
    in_maps = _prep_inputs(h, adj, W, a)
    res = run_bass_kernel_spmd(nc, in_maps, core_ids=list(range(NCORES)),
                               trace=_trace, **(_trace_kwargs or {}))
    out = np.empty((N, HO), dtype=np.float32)
    for c in range(NCORES):
        out[c * NS:(c + 1) * NS, :] = res.results[c]["out"].T
    if _trace:
        _CACHE["last_results"] = res
    return out


# revision 37
# speedup vs baseline: 1.1482x; 1.1482x over previous
"""Multi-head graph attention layer (GAT, no softmax) on 8 Trainium2 NeuronCores.

Row-shard the N=4096 nodes across the 8 cores (512 columns of P^T each).
Host precomputes Wh = h@W (bf16, [m, (h,o)] layout), the attention
projections s_h[n] = Wh.a1 and t_h[m] = Wh.a2, and the transposed,
slope-prescaled mask bigat02[m, n] = 0.2*BIG*(adj[n, m]-1).

Key factorization: in bf16, masked attention entries collapse to the
constant -0.2*BIG (~-9.007e15, matching the reference's -9e15 to ~8e-4),
so  P^T = bigat02 + prelu_0.2(s[n] + t[m])  exactly (the prelu leak on
masked entries is ~1e-15 relative).  The bigat02 term never touches a
vector engine: it enters PSUM through 4 full-width shared matmuls per
m-block on the otherwise-slack PE.  The prelu(s+t) term needs no mask
stream, so per (m-block, head) it is one op on one engine:
  - heads 0..4: ONE custom fused DVE instruction (GAT_PRELU_E) computes
    prelu(s-row-bcast + t-col-bcast) for all 5 heads (2560 elems @ 1/cyc)
  - heads 5..7: ACT Prelu with in_=s-rows, bias=t, alpha=0.2
PE accumulates the 8 per-head attention matmuls on top of the mask term
in 4 PSUM banks; elu + store at the end (split across ACT/DVE queues).

s-rows are broadcast to 128 partitions at startup via rank-1 ones
matmuls (DMA broadcast is ~10us slower); all big loads are chunked with
small leading chunks so m-block 0 starts ~6us after the preamble.
Steady state is DVE-bound at ~2.74us per m-block (custom op at its
1 elem/cycle floor); ACT runs ~2.6us, PE ~2.5us.
"""

import numpy as np
import ml_dtypes

N = 4096
IN_F = 512
OUT_F = 64
HEADS = 8
NCORES = 8
NS = N // NCORES          # 512 rows per core
MB = N // 128             # 32 m-blocks
HO = HEADS * OUT_F        # 512
BIG = float(np.float32(1.25 * 2.0**55))   # 0.2*BIG = 2^53 ~= 9.007e15
ALPHA = 0.2

# Heads 0.._NDVE-1 compute prelu(s+t) in one fused custom DVE instruction;
# the remaining heads use ACT Prelu with bias=t.  The additive-BIG mask is
# factored out entirely: P = bigat02 + prelu(s [+] t), and the bigat02 term
# goes through 4 shared full-width matmuls on the (underutilized) PE.
import os
_NDVE = int(os.environ.get("GAT_NDVE", "5"))

_CACHE = {}


def _register_op(name, spec, subdim):
    """Register a custom DVE op (idempotent), computing its uops sha."""
    import concourse.dve_ops as dops
    from concourse.dve_spec import lower, _has_src1
    from concourse.dve_uop import DveOpSpec

    if name in dops._SUB_OPCODE_FOR_NAME:
        for op in dops.OPS:
            if op.name == name:
                return op
        raise RuntimeError(f"{name} row taken but op missing")
    row = dops._CUSTOM_DVE_ROW_BASE + len(dops.OPS)
    shas = {}
    for ver in ("v3", "v4"):
        try:
            tmp = DveOpSpec(name=name, opcode=row, uops=lower(spec, ver=ver),
                            rd1_en=_has_src1(spec))
            shas[ver] = tmp.sha(ver)
        except Exception:
            pass
    op = dops.DveOp(name, spec, subdim=subdim, uops_sha=shas)
    dops.OPS.append(op)
    dops._SUB_OPCODE_FOR_NAME[name] = row
    dops.CUSTOM_DVE_SPECS[name] = spec
    return op


def _register_gat_prelu():
    """Fused prelu(in0 + in1 + s0) with slope imm2, one head-block."""
    from concourse.dve_spec import Spec, Src0, Src1, C0, C2, maxx

    def _ref(in0, in1, s0, s1, imm2):
        y = in0.astype(np.float32) + in1 + s0
        return np.maximum(y, y * imm2)

    y = Src0 + Src1 + C0
    return _register_op("GAT_PRELU_ANT", Spec(body=maxx(y, y * C2), reference=_ref),
                        subdim=False)


def _register_gat_prelu2():
    """Paired fused prelu over [P, 2, N] pages: page s gets bias s0 + s*s1
    (s0 = t of head A, s1 = t_B - t_A), slope imm2.  in1 is the concatenated
    s-rows of both heads as a flat [P, 2N] stream."""
    from concourse.dve_spec import Spec, Src0, Src1, C0, C1, C2, PageIdx, maxx

    def _ref(in0, in1, s0, s1, imm2):
        P = in0.shape[0]
        x0 = in0.astype(np.float32).reshape(P, 2, -1)
        x1 = in1.astype(np.float32).reshape(P, 2, -1)
        s0 = np.asarray(s0, np.float32).reshape(P, 1, 1)
        s1 = np.asarray(s1, np.float32).reshape(P, 1, 1)
        t = s0 + np.arange(2, dtype=np.float32)[None, :, None] * s1
        y = x0 + x1 + t
        return np.maximum(y, y * imm2).reshape(in0.shape)

    y = Src0 + Src1 + PageIdx(C0, C1)
    return _register_op("GAT_PRELU2_ANT",
                        Spec(body=maxx(y, y * C2), reference=_ref),
                        subdim=True)


def _register_gat_prelu_e():
    """prelu(in0 + in1) with slope s0: computes prelu_0.2(s[n] + t[m]) for a
    multi-head block, with in1 the per-head t column broadcast along the
    free dim.  Two tensor streams, one scalar slope."""
    from concourse.dve_spec import Spec, Src0, Src1, C0, maxx

    def _ref(in0, in1, s0, s1, imm2):
        y = in0.astype(np.float32) + in1
        return np.maximum(y, y * s0)

    y = Src0 + Src1
    return _register_op("GAT_PRELU_E_ANT",
                        Spec(body=maxx(y, y * C0), reference=_ref),
                        subdim=False)


def _build():
    import concourse.bass as bass
    import concourse.mybir as mybir
    import concourse.tile as tile
    from concourse import bacc

    gat_prelu_e = _register_gat_prelu_e()

    f32 = mybir.dt.float32
    bf16 = mybir.dt.bfloat16
    Alu = mybir.AluOpType
    Act = mybir.ActivationFunctionType

    nc = bacc.Bacc("TRN2", target_bir_lowering=False, debug=False,
                   num_devices=1)

    # host-prearranged [128, MB, x] layouts for straight contiguous DMA.
    # bigat is pre-scaled by the prelu slope: {-0.2*BIG, 0}.
    whb_d = nc.dram_tensor("whb", [128, MB, HO], bf16, kind="ExternalInput")
    bigat_d = nc.dram_tensor("bigat", [128, MB, NS], bf16,
                             kind="ExternalInput")
    srow = nc.dram_tensor("srow", [HEADS, NS], bf16, kind="ExternalInput")
    tpack = nc.dram_tensor("tpack", [128, MB, HEADS], f32,
                           kind="ExternalInput")
    tpackh = nc.dram_tensor("tpackh", [128, MB, HEADS], bf16,
                            kind="ExternalInput")
    outT = nc.dram_tensor("out", [HO, NS], f32, kind="ExternalOutput")

    nDV = _NDVE                 # heads 0..nDV-1 on the fused DVE op
    act_heads = list(range(nDV, HEADS))

    with tile.TileContext(nc) as tc:
        import contextlib
        with contextlib.ExitStack() as ctx:
            P1 = ctx.enter_context(tc.tile_pool(name="persist", bufs=1))
            pp = ctx.enter_context(tc.tile_pool(name="pp", bufs=6))
            iop = ctx.enter_context(tc.tile_pool(name="iop", bufs=2))
            hpp = ctx.enter_context(
                tc.tile_pool(name="hpp", bufs=1, space="PSUM"))

            alph = P1.tile([128, 1], f32)
            nc.gpsimd.memset(alph, ALPHA)

            # ---- upfront loads ----
            # big streaming chunks go first on the sync queue (small chunks
            # first so mb0 deps land fast); scalar-side loads ride gpsimd
            ones1 = P1.tile([1, 128], bf16)
            nc.gpsimd.memset(ones1, 1.0)
            srow1 = P1.tile([1, HEADS * NS], bf16)
            sr_ap = srow.ap()
            nc.gpsimd.dma_start(
                out=srow1,
                in_=bass.AP(tensor=sr_ap.tensor, offset=sr_ap.offset,
                            ap=[[HEADS * NS, 1], [1, HEADS * NS]]))
            whb = P1.tile([128, MB, HO], bf16)
            bigat = P1.tile([128, MB, NS], bf16)
            CHUNKS = [1, 1, 1, 1, 4, 4, 4, 4, 4, 4, 4]
            pos = 0
            for ch in CHUNKS:
                cs = slice(pos, pos + ch)
                nc.sync.dma_start(out=bigat[:, cs, :],
                                  in_=bigat_d.ap()[:, cs, :])
                nc.sync.dma_start(out=whb[:, cs, :],
                                  in_=whb_d.ap()[:, cs, :])
                pos += ch
            tsbh = P1.tile([128, MB, HEADS], bf16)  # t in bf16 (DVE in1)
            nc.gpsimd.dma_start(out=tsbh, in_=tpackh.ap())
            tsb = P1.tile([128, MB, HEADS], f32)    # t_h[m] per partition
            nc.gpsimd.dma_start(out=tsb, in_=tpack.ap())
            # s rows broadcast to 128 partitions via rank-1 ones matmuls
            # (much faster than a 128-way broadcast DMA); DVE copies the
            # fused-op heads, ACT the rest, so both start early
            sbc = P1.tile([128, HEADS, NS], bf16)   # s_h[n] bcast over parts
            with tc.tile_pool(name="bcp", bufs=4, space="PSUM") as bcp:
                for sg in range(HEADS):
                    sps = bcp.tile([128, NS], f32, tag="sps")
                    nc.tensor.matmul(sps, ones1,
                                     srow1[:, NS * sg:NS * (sg + 1)],
                                     start=True, stop=True)
                    if sg < nDV:
                        nc.vector.tensor_copy(sbc[:, sg, :], sps)
                    else:
                        nc.scalar.activation(sbc[:, sg, :], sps, Act.Prelu,
                                             bias=0.0, scale=1.0, alpha=1.0)

            # ---- PSUM accumulators: h'^T[(h,o), n], 2 heads per bank ----
            hp0 = hpp.tile([128, NS], f32, tag="hp0")
            hp1 = hpp.tile([128, NS], f32, tag="hp1")
            hp2 = hpp.tile([128, NS], f32, tag="hp2")
            hp3 = hpp.tile([128, NS], f32, tag="hp3")
            hps = [hp0, hp1, hp2, hp3]

            for mb in range(MB):
                # every 4th m-block shifts one head to ACT (it has slack)
                ndv = nDV - 1 if mb % 3 == 2 else nDV
                ah = list(range(ndv, HEADS))
                # shared mask matmuls: hps[q] (+)= whb_cols_q^T @ bigat02
                for q in range(4):
                    nc.tensor.matmul(
                        hps[q], whb[:, mb, 128 * q:128 * (q + 1)],
                        bigat[:, mb, :],
                        start=(mb == 0), stop=False,
                        skip_group_check=True)
                pc = pp.tile([128, HEADS, NS], bf16, tag="pc")
                # DVE: fused prelu(s + t) for heads 0..ndv-1, one instruction
                # (in1 = t columns broadcast along the free dim)
                tsl = tsbh[:, mb, 0:ndv]
                nc.vector._custom_dve(
                    gat_prelu_e, out=pc[:, 0:ndv, :],
                    in0=sbc[:, 0:ndv, :],
                    in1=bass.AP(tensor=tsl.tensor, offset=tsl.offset,
                                ap=[tsl.ap[0], [tsl.ap[-1][0], ndv],
                                    [0, NS]]),
                    s0=ALPHA)
                # ACT: prelu(s + t) via bias for the rest
                for hh in ah:
                    nc.scalar.activation(pc[:, hh, :], sbc[:, hh, :],
                                         Act.Prelu,
                                         bias=tsb[:, mb, hh:hh + 1],
                                         scale=1.0, alpha=alph[:, 0:1])
                # PE: per-head attention matmuls (accumulate onto mask term)
                for hh in list(range(ndv)) + ah:
                    po = 64 * (hh % 2)
                    nc.tensor.matmul(
                        hps[hh // 2][po:po + 64, :],
                        whb[:, mb, OUT_F * hh:OUT_F * (hh + 1)],
                        pc[:, hh, :],
                        start=False, stop=(mb == MB - 1),
                        skip_group_check=True)

            # ---- output: elu, store transposed (host untransposes).
            # q0/q1 chains lean on ACT, q2/q3 on DVE, so the tails overlap.
            for q in range(4):
                rpos = iop.tile([128, NS], f32, tag=f"rpos{q}")
                rneg = iop.tile([128, NS], f32, tag=f"rneg{q}")
                ex = iop.tile([128, NS], f32, tag=f"ex{q}")
                if q < 2:
                    nc.scalar.activation(rpos, hps[q], Act.Relu)
                    nc.scalar.activation(rneg, hps[q], Act.Relu, scale=-1.0)
                    nc.scalar.activation(ex, rneg, Act.Exp, scale=-1.0)
                else:
                    nc.vector.tensor_scalar(rpos, hps[q], 0.0, None, Alu.max)
                    nc.vector.tensor_scalar(rneg, hps[q], 0.0, None, Alu.min)
                    nc.scalar.activation(ex, rneg, Act.Exp)
                oo = iop.tile([128, NS], f32, tag=f"oo{q}")
                nc.vector.scalar_tensor_tensor(
                    out=oo, in0=rpos, scalar=-1.0, in1=ex,
                    op0=Alu.add, op1=Alu.add)
                engs = [nc.sync, nc.gpsimd, nc.scalar]
                th = NS // 3 + 1
                for pz in range(3):
                    lo, hi = th * pz, min(th * (pz + 1), NS)
                    engs[(q + pz) % 3].dma_start(
                        out=outT.ap()[128 * q:128 * (q + 1), lo:hi],
                        in_=oo[:, lo:hi])

    nc.compile()
    return nc


def _prep_inputs(h, adj, W, a):
    bf = ml_dtypes.bfloat16
    # Wh[h, n, o] then column-major (h,o) concat -> [n, 64h+o]
    Wh = np.matmul(h[None, :, :], W)                       # [H, N, O] f32
    whb_no = Wh.transpose(1, 0, 2).reshape(N, HO)          # [N, HO]
    whb = np.ascontiguousarray(
        whb_no.reshape(MB, 128, HO).transpose(1, 0, 2)).astype(bf)
    a1 = a[:, :OUT_F, 0]                                   # [H, O] (s side)
    a2 = a[:, OUT_F:, 0]                                   # [H, O] (t side)
    s_full = np.matmul(Wh, a1[:, :, None])[:, :, 0]        # [H, N]
    t_full = np.matmul(Wh, a2[:, :, None])[:, :, 0]        # [H, N]
    tpack = np.ascontiguousarray(
        t_full.T.reshape(MB, 128, HEADS).transpose(1, 0, 2)).astype(
            np.float32)                                    # [128, MB, H]
    # mask pre-scaled by the prelu slope: {-0.2*BIG, 0}
    bigaT = ((adj.T.astype(np.float32) - 1.0) * (ALPHA * BIG)).astype(bf)

    in_maps = []
    for c in range(NCORES):
        rows = slice(c * NS, (c + 1) * NS)
        bslice = np.ascontiguousarray(
            bigaT[:, rows].reshape(MB, 128, NS).transpose(1, 0, 2))
        in_maps.append({
            "whb": whb,
            "bigat": bslice,
            "srow": np.ascontiguousarray(s_full[:, rows]).astype(bf),
            "tpack": tpack,
            "tpackh": tpack.astype(bf),
        })
    return in_maps


def _get_nc():
    if "nc" not in _CACHE:
        _CACHE["nc"] = _build()
    return _CACHE["nc"]


def kernel(h, adj, W, a, _trace=False, _trace_kwargs=None):
    from concourse.bass_utils import run_bass_kernel_spmd

    h = np.asarray(h, dtype=np.float32)
    adj = np.asarray(adj, dtype=np.int32)
    W = np.asarray(W, dtype=np.float32)
    a = np.asarray(a, dtype=np.float32)

    nc = _get_nc()
    in_maps = _prep_inputs(h, adj, W, a)
    res = run_bass_kernel_spmd(nc, in_maps, core_ids=list(range(NCORES)),
                               trace=_trace, **(_trace_kwargs or {}))
    out = np.empty((N, HO), dtype=np.float32)
    for c in range(NCORES):
        out[c * NS:(c + 1) * NS, :] = res.results[c]["out"].T
    if _trace:
        _CACHE["last_results"] = res
    return out
